# revision 7
# baseline (speedup 1.0000x reference)
"""Trainium2 Bass kernel for BatchNorm2d + 8-head self-attention block.

Reference (per batch element b, all fp32):
    xn = BN_eval(x[b]); t = xn.T
    q/k/v = t @ W.T + b            # [S, 512], 8 heads x 64
    attn  = softmax(q k^T / 8)     # per head
    y[b]  = ((attn v) @ wo.T + bo).T

Sharding: pure data parallel — one batch element per NeuronCore, weights
replicated, no collectives.

Device design (per core), fully in the "transposed" domain (no large
transposes anywhere):
  - BN folded into QKV weights/biases on host; 1/8 scale folded into wq/bq;
    v bias folded into bo (softmax rows sum to 1).
  - Q^T,K^T [I,S] = wT.T @ x      (x arrives [C,S] — natural rhs)
  - V [S,I]       = x_chunk.T @ wvT, stored interleaved per head with a
    ones column ([128, 8*65]) so the PV matmul (M=65) also produces the
    softmax denominators for free.
  - scores^T per head [t,s]; head pairs row-packed via tile_position
    (0,0)/(64,0), K=64 each; exp on ScalarE over both heads in one call
    (no max subtraction — BN-normalized inputs keep scores small).
  - o^T accumulates over 8 t-chunks (K=128); normalize = approx-reciprocal
    row + gpsimd partition-broadcast + DVE multiply; y^T = woT.T @ o^T + bo.
All matmuls in fp32r (full PE rate at N=512, ~1e-4 relative error).
"""

import numpy as np

import concourse.bass as bass
import concourse.tile as tile
from concourse import bacc, mybir
from concourse.bass_utils import run_bass_kernel_spmd

B, C, S = 8, 512, 1024
H, DH, INNER = 8, 64, 512
EPS = 1e-5
SCALE = DH ** (-0.5)
N_CORES = 8
F32 = mybir.dt.float32
F32R = mybir.dt.float32r

_CACHE: dict = {}

KC = C // 128      # 4 contraction chunks over channels
IT = INNER // 128  # 4 tiles over inner dim (also head-pair index)
ST = S // 128      # 8 t-chunks
NSLAB = S // 512   # 2 s-slabs


def build_bass():
    nc = bacc.Bacc("TRN2", target_bir_lowering=False, debug=False,
                   num_devices=N_CORES)

    x_d = nc.dram_tensor("x", [C, S], F32, kind="ExternalInput")
    wqT_d = nc.dram_tensor("wqT", [C, INNER], F32, kind="ExternalInput")
    wkT_d = nc.dram_tensor("wkT", [C, INNER], F32, kind="ExternalInput")
    wvT_d = nc.dram_tensor("wvT", [C, INNER], F32, kind="ExternalInput")
    woT_d = nc.dram_tensor("woT", [INNER, C], F32, kind="ExternalInput")
    bq_d = nc.dram_tensor("bq", [INNER], F32, kind="ExternalInput")
    bk_d = nc.dram_tensor("bk", [INNER], F32, kind="ExternalInput")
    bo_d = nc.dram_tensor("bo", [C], F32, kind="ExternalInput")
    y_d = nc.dram_tensor("y", [C, S], F32, kind="ExternalOutput")

    with tile.TileContext(nc) as tc:
        with (
            tc.tile_pool(name="persist", bufs=1) as persist,
            tc.tile_pool(name="stage", bufs=2) as stage,
            tc.tile_pool(name="out", bufs=3) as outp,
            tc.tile_pool(name="et", bufs=4) as etp,
            tc.tile_pool(name="norm", bufs=2) as normp,
            tc.tile_pool(name="psP", bufs=2, space="PSUM") as psP,
            tc.tile_pool(name="psS", bufs=2, space="PSUM") as psS,
            tc.tile_pool(name="psO", bufs=1, space="PSUM") as psO,
        ):
            # ---- loads. Priority: wq first (alone on the sync/HWDGE queue,
            # so it lands in ~3us and the first projections start ASAP), the
            # remaining weights chained behind it in need-order (wv before wk:
            # attention needs V ready early). x + tiny biases ride the gpsimd
            # (SWDGE) queue in parallel. NOTHING on the scalar queue — the
            # scalar engine must stay exp-only. ----
            from concourse.tile import add_dep_helper

            wq_st = stage.tile([128, KC, 512], F32, tag="wq_st", bufs=1)
            i_wq = nc.sync.dma_start(
                wq_st[:], wqT_d.rearrange("(k p) i -> p k i", p=128))
            wv_st = stage.tile([128, KC, 512], F32, tag="wv_st", bufs=1)
            i_wv = nc.sync.dma_start(
                wv_st[:], wvT_d.rearrange("(k p) i -> p k i", p=128))
            wk_st = stage.tile([128, KC, 512], F32, tag="wk_st", bufs=1)
            i_wk = nc.sync.dma_start(
                wk_st[:], wkT_d.rearrange("(k p) i -> p k i", p=128))
            wo_st = stage.tile([128, KC, 512], F32, tag="wo_st", bufs=1)
            i_wo = nc.sync.dma_start(
                wo_st[:], woT_d.rearrange("(k p) i -> p k i", p=128))
            add_dep_helper(i_wv.ins, i_wq.ins, sync=False, reason="dma priority")
            add_dep_helper(i_wk.ins, i_wv.ins, sync=False, reason="dma priority")
            add_dep_helper(i_wo.ins, i_wk.ins, sync=False, reason="dma priority")

            bq_sb = persist.tile([128, IT], F32, tag="bq")
            nc.gpsimd.dma_start(bq_sb[:], bq_d.rearrange("(t p) -> p t", p=128))
            bk_sb = persist.tile([128, IT], F32, tag="bk")
            nc.gpsimd.dma_start(bk_sb[:], bk_d.rearrange("(t p) -> p t", p=128))
            bo_sb = persist.tile([128, IT], F32, tag="bo")
            nc.gpsimd.dma_start(bo_sb[:], bo_d.rearrange("(t p) -> p t", p=128))
            x_st = stage.tile([128, KC, S], F32, tag="x_st", bufs=1)
            x_r3 = x_d.rearrange("(k p) s -> p k s", p=128)
            nc.gpsimd.dma_start(x_st[:, 0:2, :], x_r3[:, 0:2, :])
            nc.gpsimd.dma_start(x_st[:, 2:4, :], x_r3[:, 2:4, :])

            # ---- casts to fp32r ----
            xr = persist.tile([128, KC, S], F32R, tag="xr", name="xr")
            for kc in range(KC):
                nc.vector.tensor_copy(xr[:, kc, :], x_st[:, kc, :])
            wqr = persist.tile([128, KC, 512], F32R, tag="wqr", name="wqr")
            nc.vector.tensor_copy(wqr[:], wq_st[:])
            wkr = persist.tile([128, KC, 512], F32R, tag="wkr", name="wkr")
            nc.vector.tensor_copy(wkr[:], wk_st[:])
            wvr = persist.tile([128, KC, 512], F32R, tag="wvr", name="wvr")
            nc.vector.tensor_copy(wvr[:], wv_st[:])
            wor = persist.tile([128, KC, 512], F32R, tag="wor", name="wor")
            nc.vector.tensor_copy(wor[:], wo_st[:])

            ones_sb = persist.tile([128, H], F32, tag="ones")
            nc.vector.memset(ones_sb[:], 1.0)

            # ---- persistent per-slab outputs ----
            qT = [[persist.tile([128, 512], F32R, tag=f"qT{i}{s}",
                                name=f"qT{i}{s}") for s in range(NSLAB)]
                  for i in range(IT)]
            kT = [[persist.tile([128, 512], F32R, tag=f"kT{i}{s}",
                                name=f"kT{i}{s}") for s in range(NSLAB)]
                  for i in range(IT)]
            oT = [[persist.tile([128, 512], F32R, tag=f"oT{i}{s}",
                                name=f"oT{i}{s}") for s in range(NSLAB)]
                  for i in range(IT)]
            v_sb = [persist.tile([128, H * 65], F32R, tag=f"v{t}",
                                 name=f"v{t}") for t in range(ST)]

            def qk_proj(hp):
                for w, bias, dst in ((wqr, bq_sb, qT), (wkr, bk_sb, kT)):
                    for sl in range(NSLAB):
                        ps = psP.tile([128, 512], F32, tag="psP", name="psP")
                        for kc in range(KC):
                            nc.tensor.matmul(
                                ps[:],
                                w[:, kc, hp * 128:(hp + 1) * 128],
                                xr[:, kc, sl * 512:(sl + 1) * 512],
                                start=(kc == 0), stop=(kc == KC - 1),
                            )
                        nc.vector.tensor_scalar_add(
                            dst[hp][sl][:], ps[:], bias[:, hp:hp + 1]
                        )

            def v_proj(tc_):
                ps = psP.tile([128, 512], F32, tag="psP", name="psP")
                for kc in range(KC):
                    nc.tensor.matmul(
                        ps[:],
                        xr[:, kc, tc_ * 128:(tc_ + 1) * 128],
                        wvr[:, kc, :],
                        start=(kc == 0), stop=(kc == KC - 1),
                    )
                vv = v_sb[tc_][:].rearrange("p (h m) -> p h m", h=H)
                nc.vector.tensor_copy(
                    vv[:, :, 0:64], ps[:].rearrange("p (h m) -> p h m", h=H)
                )
                nc.vector.tensor_copy(vv[:, :, 64:65], ones_sb[:, :, None])

            def attention(sl, hp):
                h0, h1 = 2 * hp, 2 * hp + 1
                po0 = psO.tile([65, 512], F32, tag="po0", name="po0")
                po1 = psO.tile([65, 512], F32, tag="po1", name="po1")
                for tc_ in range(ST):
                    ksl, kcol = tc_ // 4, (tc_ % 4) * 128
                    pss = psS.tile([128, 1024], F32, tag="psS", name="psS")
                    nc.tensor.matmul(
                        pss[:, 0:512],
                        kT[hp][ksl][0:64, kcol:kcol + 128],
                        qT[hp][sl][0:64, :],
                        start=True, stop=True, tile_position=(0, 0),
                    )
                    nc.tensor.matmul(
                        pss[:, 512:1024],
                        kT[hp][ksl][64:128, kcol:kcol + 128],
                        qT[hp][sl][64:128, :],
                        start=True, stop=True, tile_position=(64, 0),
                    )
                    et = etp.tile([128, 1024], F32R, tag="et", name="et")
                    nc.scalar.activation(
                        et[:], pss[:], mybir.ActivationFunctionType.Exp
                    )
                    nc.tensor.matmul(
                        po0[:], v_sb[tc_][:, h0 * 65:(h0 + 1) * 65],
                        et[:, 0:512],
                        start=(tc_ == 0), stop=(tc_ == ST - 1),
                    )
                    nc.tensor.matmul(
                        po1[:], v_sb[tc_][:, h1 * 65:(h1 + 1) * 65],
                        et[:, 512:1024],
                        start=(tc_ == 0), stop=(tc_ == ST - 1),
                    )
                for half, po in ((0, po0), (1, po1)):
                    drow = normp.tile([1, 512], F32, tag="drow", name="drow")
                    nc.vector.tensor_copy(drow[:], po[64:65, :])
                    rrow = normp.tile([1, 512], F32, tag="rrow", name="rrow")
                    nc.vector.reciprocal_approx_fast(rrow[:], drow[:])
                    rbc = normp.tile([64, 512], F32, tag="rbc", name="rbc")
                    nc.gpsimd.partition_broadcast(rbc[:], rrow[:])
                    nc.vector.tensor_mul(
                        oT[hp][sl][half * 64:(half + 1) * 64, :],
                        po[0:64, :],
                        rbc[:],
                    )

            def out_proj(sl):
                for ct in range(IT):
                    ps = psP.tile([128, 512], F32, tag="psP", name="psP")
                    for ic in range(IT):
                        nc.tensor.matmul(
                            ps[:],
                            wor[:, ic, ct * 128:(ct + 1) * 128],
                            oT[ic][sl][:],
                            start=(ic == 0), stop=(ic == IT - 1),
                        )
                    ysb = outp.tile([128, 512], F32, tag="ysb", name="ysb")
                    nc.vector.tensor_scalar_add(ysb[:], ps[:], bo_sb[:, ct:ct + 1])
                    nc.sync.dma_start(
                        y_d[ct * 128:(ct + 1) * 128, sl * 512:(sl + 1) * 512],
                        ysb[:],
                    )

            # ---- emission order (priority hint for the scheduler) ----
            qk_proj(0)
            for tc_ in range(ST):
                v_proj(tc_)
            emitted = {0}
            for sl in range(NSLAB):
                for hp in range(IT):
                    if hp not in emitted:
                        qk_proj(hp)
                        emitted.add(hp)
                    attention(sl, hp)
                out_proj(sl)

    nc.compile()
    return nc


def prep_host(inputs):
    """Fold BN + scale + v-bias into effective weights (fp32 numpy)."""
    x = np.asarray(inputs["x"], dtype=np.float32)
    g = np.asarray(inputs["bn_gamma"], dtype=np.float32)
    be = np.asarray(inputs["bn_beta"], dtype=np.float32)
    mu = np.asarray(inputs["bn_mean"], dtype=np.float32)
    var = np.asarray(inputs["bn_var"], dtype=np.float32)
    wq = np.asarray(inputs["wq"], dtype=np.float32)
    bq = np.asarray(inputs["bq"], dtype=np.float32)
    wk = np.asarray(inputs["wk"], dtype=np.float32)
    bk = np.asarray(inputs["bk"], dtype=np.float32)
    wv = np.asarray(inputs["wv"], dtype=np.float32)
    bv = np.asarray(inputs["bv"], dtype=np.float32)
    wo = np.asarray(inputs["wo"], dtype=np.float32)
    bo = np.asarray(inputs["bo"], dtype=np.float32)

    a = g / np.sqrt(var + EPS)          # [C]
    bvec = be - mu * a                  # [C]

    wq_eff = wq * a[None, :] * SCALE
    bq_eff = (bq + wq @ bvec) * SCALE
    wk_eff = wk * a[None, :]
    bk_eff = bk + wk @ bvec
    wv_eff = wv * a[None, :]
    bv_eff = bv + wv @ bvec
    bo_eff = bo + wo @ bv_eff           # v bias rides through softmax (sums to 1)

    per_core = []
    for b in range(B):
        per_core.append({
            "x": np.ascontiguousarray(x[b, :, :, 0]),
            "wqT": np.ascontiguousarray(wq_eff.T),
            "wkT": np.ascontiguousarray(wk_eff.T),
            "wvT": np.ascontiguousarray(wv_eff.T),
            "woT": np.ascontiguousarray(wo.T),
            "bq": bq_eff,
            "bk": bk_eff,
            "bo": bo_eff,
        })
    return per_core


def kernel(**inputs):
    if "nc" not in _CACHE:
        _CACHE["nc"] = build_bass()
    nc = _CACHE["nc"]
    in_maps = prep_host(inputs)
    res = run_bass_kernel_spmd(nc, in_maps, list(range(N_CORES)))
    y = np.stack([res.results[c]["y"] for c in range(N_CORES)], axis=0)
    return y[..., None].astype(np.float32)


def run_traced(**inputs):
    """Like kernel() but with NTFF profiling; returns (y, results, tmpdir)."""
    if "nc" not in _CACHE:
        _CACHE["nc"] = build_bass()
    nc = _CACHE["nc"]
    in_maps = prep_host(inputs)
    import tempfile
    tmpdir = tempfile.mkdtemp(prefix="mha_trace_")
    res = run_bass_kernel_spmd(
        nc, in_maps, list(range(N_CORES)), trace=True, tmpdir=tmpdir
    )
    y = np.stack([res.results[c]["y"] for c in range(N_CORES)], axis=0)
    return y[..., None].astype(np.float32), res, tmpdir


# revision 11
# speedup vs baseline: 1.0285x; 1.0285x over previous
"""Trainium2 Bass kernel for BatchNorm2d + 8-head self-attention block.

Reference (per batch element b, all fp32):
    xn = BN_eval(x[b]); t = xn.T
    q/k/v = t @ W.T + b            # [S, 512], 8 heads x 64
    attn  = softmax(q k^T / 8)     # per head
    y[b]  = ((attn v) @ wo.T + bo).T

Sharding: pure data parallel — one batch element per NeuronCore, weights
replicated, no collectives.

Device design (per core), fully in the "transposed" domain (no large
transposes anywhere):
  - BN folded into QKV weights/biases on host; 1/8 scale folded into wq/bq;
    v bias folded into bo (softmax rows sum to 1).
  - Q^T,K^T [I,S] = wT.T @ x      (x arrives [C,S] — natural rhs)
  - V [S,I]       = x_chunk.T @ wvT, stored interleaved per head with a
    ones column ([128, 8*65]) so the PV matmul (M=65) also produces the
    softmax denominators for free.
  - scores^T per head [t,s]; head pairs row-packed via tile_position
    (0,0)/(64,0), K=64 each; exp on ScalarE over both heads in one call
    (no max subtraction — BN-normalized inputs keep scores small).
  - o^T accumulates over 8 t-chunks (K=128); normalize = approx-reciprocal
    row + gpsimd partition-broadcast + DVE multiply; y^T = woT.T @ o^T + bo.
All matmuls in fp32r (full PE rate at N=512, ~1e-4 relative error).
"""

import numpy as np

import concourse.bass as bass
import concourse.tile as tile
from concourse import bacc, mybir
from concourse.bass_utils import run_bass_kernel_spmd

B, C, S = 8, 512, 1024
H, DH, INNER = 8, 64, 512
EPS = 1e-5
SCALE = DH ** (-0.5)
N_CORES = 8
F32 = mybir.dt.float32
F32R = mybir.dt.float32r

_CACHE: dict = {}

KC = C // 128      # 4 contraction chunks over channels
IT = INNER // 128  # 4 tiles over inner dim (also head-pair index)
ST = S // 128      # 8 t-chunks
NSLAB = S // 512   # 2 s-slabs


def build_bass():
    nc = bacc.Bacc("TRN2", target_bir_lowering=False, debug=False,
                   num_devices=N_CORES)

    x_d = nc.dram_tensor("x", [C, S], F32, kind="ExternalInput")
    wqT_d = nc.dram_tensor("wqT", [C, INNER], F32, kind="ExternalInput")
    wkT_d = nc.dram_tensor("wkT", [C, INNER], F32, kind="ExternalInput")
    wvT_d = nc.dram_tensor("wvT", [C, INNER], F32, kind="ExternalInput")
    woT_d = nc.dram_tensor("woT", [INNER, C], F32, kind="ExternalInput")
    # bq | bk | bo packed on host as [128, 12] (col t+0/4/8 = vec[t*128+p])
    bias_d = nc.dram_tensor("bias_pack", [128, 3 * IT], F32, kind="ExternalInput")
    y_d = nc.dram_tensor("y", [C, S], F32, kind="ExternalOutput")

    with tile.TileContext(nc) as tc:
        with (
            tc.tile_pool(name="persist", bufs=1) as persist,
            tc.tile_pool(name="stage", bufs=2) as stage,
            tc.tile_pool(name="out", bufs=3) as outp,
            tc.tile_pool(name="et", bufs=4) as etp,
            tc.tile_pool(name="norm", bufs=2) as normp,
            tc.tile_pool(name="psP", bufs=2, space="PSUM") as psP,
            tc.tile_pool(name="psS", bufs=2, space="PSUM") as psS,
            tc.tile_pool(name="psO", bufs=1, space="PSUM") as psO,
        ):
            # ---- loads. Priority: wq first (alone on the sync/HWDGE queue,
            # so it lands in ~3us and the first projections start ASAP), the
            # remaining weights chained behind it in need-order (wv before wk:
            # attention needs V ready early). x + tiny biases ride the gpsimd
            # (SWDGE) queue in parallel. NOTHING on the scalar queue — the
            # scalar engine must stay exp-only. ----
            from concourse.tile import add_dep_helper

            # x chunks first (each its own tile so the casts pipeline), then
            # weights in need-order, all chained on the sync/HWDGE queue so
            # bandwidth goes to the critical transfer instead of round-robin.
            x_st = [stage.tile([128, S], F32, tag=f"x_st{k}", name=f"x_st{k}",
                               bufs=1) for k in range(KC)]
            prev = None
            dmas = []
            for kc in range(KC):
                i = nc.sync.dma_start(x_st[kc][:], x_d[kc * 128:(kc + 1) * 128, :])
                dmas.append(i)
            wq_st = stage.tile([128, KC, 512], F32, tag="wq_st", bufs=1)
            dmas.append(nc.sync.dma_start(
                wq_st[:], wqT_d.rearrange("(k p) i -> p k i", p=128)))
            wk_st = stage.tile([128, KC, 512], F32, tag="wk_st", bufs=1)
            dmas.append(nc.sync.dma_start(
                wk_st[:], wkT_d.rearrange("(k p) i -> p k i", p=128)))
            wv_st = stage.tile([128, KC, 512], F32, tag="wv_st", bufs=1)
            dmas.append(nc.sync.dma_start(
                wv_st[:], wvT_d.rearrange("(k p) i -> p k i", p=128)))
            wo_st = stage.tile([128, KC, 512], F32, tag="wo_st", bufs=1)
            dmas.append(nc.sync.dma_start(
                wo_st[:], woT_d.rearrange("(k p) i -> p k i", p=128)))
            for a, b in zip(dmas[1:], dmas):
                add_dep_helper(a.ins, b.ins, sync=False, reason="dma priority")

            bias_sb = persist.tile([128, 3 * IT], F32, tag="bias")
            nc.gpsimd.dma_start(bias_sb[:], bias_d[:])
            bq_sb = bias_sb[:, 0:IT]
            bk_sb = bias_sb[:, IT:2 * IT]
            bo_sb = bias_sb[:, 2 * IT:3 * IT]

            # ---- casts to fp32r ----
            xr = persist.tile([128, KC, S], F32R, tag="xr", name="xr")
            for kc in range(KC):
                nc.vector.tensor_copy(xr[:, kc, :], x_st[kc][:])
            wqr = persist.tile([128, KC, 512], F32R, tag="wqr", name="wqr")
            nc.vector.tensor_copy(wqr[:], wq_st[:])
            wkr = persist.tile([128, KC, 512], F32R, tag="wkr", name="wkr")
            nc.vector.tensor_copy(wkr[:], wk_st[:])
            wvr = persist.tile([128, KC, 512], F32R, tag="wvr", name="wvr")
            nc.vector.tensor_copy(wvr[:], wv_st[:])
            wor = persist.tile([128, KC, 512], F32R, tag="wor", name="wor")
            nc.vector.tensor_copy(wor[:], wo_st[:])

            ones_sb = persist.tile([128, H], F32, tag="ones")
            nc.vector.memset(ones_sb[:], 1.0)

            # ---- persistent per-slab outputs ----
            qT = [[persist.tile([128, 512], F32R, tag=f"qT{i}{s}",
                                name=f"qT{i}{s}") for s in range(NSLAB)]
                  for i in range(IT)]
            kT = [[persist.tile([128, 512], F32R, tag=f"kT{i}{s}",
                                name=f"kT{i}{s}") for s in range(NSLAB)]
                  for i in range(IT)]
            oT = [[persist.tile([128, 512], F32R, tag=f"oT{i}{s}",
                                name=f"oT{i}{s}") for s in range(NSLAB)]
                  for i in range(IT)]
            v_sb = [persist.tile([128, H * 65], F32R, tag=f"v{t}",
                                 name=f"v{t}") for t in range(ST)]

            def qk_proj(hp):
                for w, bias, dst in ((wqr, bq_sb, qT), (wkr, bk_sb, kT)):
                    for sl in range(NSLAB):
                        ps = psP.tile([128, 512], F32, tag="psP", name="psP")
                        for kc in range(KC):
                            nc.tensor.matmul(
                                ps[:],
                                w[:, kc, hp * 128:(hp + 1) * 128],
                                xr[:, kc, sl * 512:(sl + 1) * 512],
                                start=(kc == 0), stop=(kc == KC - 1),
                            )
                        nc.vector.tensor_scalar_add(
                            dst[hp][sl][:], ps[:], bias[:, hp:hp + 1]
                        )

            def v_proj(tc_):
                ps = psP.tile([128, 512], F32, tag="psP", name="psP")
                for kc in range(KC):
                    nc.tensor.matmul(
                        ps[:],
                        xr[:, kc, tc_ * 128:(tc_ + 1) * 128],
                        wvr[:, kc, :],
                        start=(kc == 0), stop=(kc == KC - 1),
                    )
                vv = v_sb[tc_][:].rearrange("p (h m) -> p h m", h=H)
                nc.vector.tensor_copy(
                    vv[:, :, 0:64], ps[:].rearrange("p (h m) -> p h m", h=H)
                )
                nc.vector.tensor_copy(vv[:, :, 64:65], ones_sb[:, :, None])

            def attention(sl, hp):
                h0, h1 = 2 * hp, 2 * hp + 1
                po0 = psO.tile([65, 512], F32, tag="po0", name="po0")
                po1 = psO.tile([65, 512], F32, tag="po1", name="po1")
                for tc_ in range(ST):
                    ksl, kcol = tc_ // 4, (tc_ % 4) * 128
                    pss = psS.tile([128, 1024], F32, tag="psS", name="psS")
                    nc.tensor.matmul(
                        pss[:, 0:512],
                        kT[hp][ksl][0:64, kcol:kcol + 128],
                        qT[hp][sl][0:64, :],
                        start=True, stop=True, tile_position=(0, 0),
                    )
                    nc.tensor.matmul(
                        pss[:, 512:1024],
                        kT[hp][ksl][64:128, kcol:kcol + 128],
                        qT[hp][sl][64:128, :],
                        start=True, stop=True, tile_position=(64, 0),
                    )
                    et = etp.tile([128, 1024], F32R, tag="et", name="et")
                    nc.scalar.activation(
                        et[:], pss[:], mybir.ActivationFunctionType.Exp
                    )
                    nc.tensor.matmul(
                        po0[:], v_sb[tc_][:, h0 * 65:(h0 + 1) * 65],
                        et[:, 0:512],
                        start=(tc_ == 0), stop=(tc_ == ST - 1),
                    )
                    nc.tensor.matmul(
                        po1[:], v_sb[tc_][:, h1 * 65:(h1 + 1) * 65],
                        et[:, 512:1024],
                        start=(tc_ == 0), stop=(tc_ == ST - 1),
                    )
                for half, po in ((0, po0), (1, po1)):
                    drow = normp.tile([1, 512], F32, tag="drow", name="drow")
                    nc.vector.tensor_copy(drow[:], po[64:65, :])
                    rrow = normp.tile([1, 512], F32, tag="rrow", name="rrow")
                    nc.vector.reciprocal_approx_fast(rrow[:], drow[:])
                    rbc = normp.tile([64, 512], F32, tag="rbc", name="rbc")
                    nc.gpsimd.partition_broadcast(rbc[:], rrow[:])
                    nc.vector.tensor_mul(
                        oT[hp][sl][half * 64:(half + 1) * 64, :],
                        po[0:64, :],
                        rbc[:],
                    )

            def out_proj(sl):
                for ct in range(IT):
                    ps = psP.tile([128, 512], F32, tag="psP", name="psP")
                    for ic in range(IT):
                        nc.tensor.matmul(
                            ps[:],
                            wor[:, ic, ct * 128:(ct + 1) * 128],
                            oT[ic][sl][:],
                            start=(ic == 0), stop=(ic == IT - 1),
                        )
                    ysb = outp.tile([128, 512], F32, tag="ysb", name="ysb")
                    nc.vector.tensor_scalar_add(ysb[:], ps[:], bo_sb[:, ct:ct + 1])
                    nc.sync.dma_start(
                        y_d[ct * 128:(ct + 1) * 128, sl * 512:(sl + 1) * 512],
                        ysb[:],
                    )

            # ---- emission order (priority hint for the scheduler) ----
            qk_proj(0)
            for tc_ in range(ST):
                v_proj(tc_)
            emitted = {0}
            for sl in range(NSLAB):
                for hp in range(IT):
                    if hp not in emitted:
                        qk_proj(hp)
                        emitted.add(hp)
                    attention(sl, hp)
                out_proj(sl)

    nc.compile()
    return nc


def prep_host(inputs):
    """Fold BN + scale + v-bias into effective weights (fp32 numpy)."""
    x = np.asarray(inputs["x"], dtype=np.float32)
    g = np.asarray(inputs["bn_gamma"], dtype=np.float32)
    be = np.asarray(inputs["bn_beta"], dtype=np.float32)
    mu = np.asarray(inputs["bn_mean"], dtype=np.float32)
    var = np.asarray(inputs["bn_var"], dtype=np.float32)
    wq = np.asarray(inputs["wq"], dtype=np.float32)
    bq = np.asarray(inputs["bq"], dtype=np.float32)
    wk = np.asarray(inputs["wk"], dtype=np.float32)
    bk = np.asarray(inputs["bk"], dtype=np.float32)
    wv = np.asarray(inputs["wv"], dtype=np.float32)
    bv = np.asarray(inputs["bv"], dtype=np.float32)
    wo = np.asarray(inputs["wo"], dtype=np.float32)
    bo = np.asarray(inputs["bo"], dtype=np.float32)

    a = g / np.sqrt(var + EPS)          # [C]
    bvec = be - mu * a                  # [C]

    wq_eff = wq * a[None, :] * SCALE
    bq_eff = (bq + wq @ bvec) * SCALE
    wk_eff = wk * a[None, :]
    bk_eff = bk + wk @ bvec
    wv_eff = wv * a[None, :]
    bv_eff = bv + wv @ bvec
    bo_eff = bo + wo @ bv_eff           # v bias rides through softmax (sums to 1)

    bias_pack = np.concatenate(
        [bq_eff.reshape(IT, 128).T, bk_eff.reshape(IT, 128).T,
         bo_eff.reshape(IT, 128).T], axis=1
    ).astype(np.float32)

    per_core = []
    for b in range(B):
        per_core.append({
            "x": np.ascontiguousarray(x[b, :, :, 0]),
            "wqT": np.ascontiguousarray(wq_eff.T),
            "wkT": np.ascontiguousarray(wk_eff.T),
            "wvT": np.ascontiguousarray(wv_eff.T),
            "woT": np.ascontiguousarray(wo.T),
            "bias_pack": np.ascontiguousarray(bias_pack),
        })
    return per_core


def kernel(**inputs):
    if "nc" not in _CACHE:
        _CACHE["nc"] = build_bass()
    nc = _CACHE["nc"]
    in_maps = prep_host(inputs)
    res = run_bass_kernel_spmd(nc, in_maps, list(range(N_CORES)))
    y = np.stack([res.results[c]["y"] for c in range(N_CORES)], axis=0)
    return y[..., None].astype(np.float32)


def run_traced(**inputs):
    """Like kernel() but with NTFF profiling; returns (y, results, tmpdir)."""
    if "nc" not in _CACHE:
        _CACHE["nc"] = build_bass()
    nc = _CACHE["nc"]
    in_maps = prep_host(inputs)
    import tempfile
    tmpdir = tempfile.mkdtemp(prefix="mha_trace_")
    res = run_bass_kernel_spmd(
        nc, in_maps, list(range(N_CORES)), trace=True, tmpdir=tmpdir
    )
    y = np.stack([res.results[c]["y"] for c in range(N_CORES)], axis=0)
    return y[..., None].astype(np.float32), res, tmpdir


# revision 16
# speedup vs baseline: 1.0432x; 1.0142x over previous
"""Trainium2 Bass kernel for BatchNorm2d + 8-head self-attention block.

Reference (per batch element b, all fp32):
    xn = BN_eval(x[b]); t = xn.T
    q/k/v = t @ W.T + b            # [S, 512], 8 heads x 64
    attn  = softmax(q k^T / 8)     # per head
    y[b]  = ((attn v) @ wo.T + bo).T

Sharding: pure data parallel — one batch element per NeuronCore, weights
replicated, no collectives.

Device design (per core), fully in the "transposed" domain (no large
transposes anywhere):
  - BN folded into QKV weights/biases on host; 1/8 scale folded into wq/bq;
    v bias folded into bo (softmax rows sum to 1).
  - Q^T,K^T [I,S] = wT.T @ x      (x arrives [C,S] — natural rhs)
  - V [S,I]       = x_chunk.T @ wvT, stored interleaved per head with a
    ones column ([128, 8*65]) so the PV matmul (M=65) also produces the
    softmax denominators for free.
  - scores^T per head [t,s]; head pairs row-packed via tile_position
    (0,0)/(64,0), K=64 each; exp on ScalarE over both heads in one call
    (no max subtraction — BN-normalized inputs keep scores small).
  - o^T accumulates over 8 t-chunks (K=128); normalize = approx-reciprocal
    row + gpsimd partition-broadcast + DVE multiply; y^T = woT.T @ o^T + bo.
All matmuls in fp32r (full PE rate at N=512, ~1e-4 relative error).
"""

import numpy as np

import concourse.bass as bass
import concourse.tile as tile
from concourse import bacc, mybir
from concourse.bass_utils import run_bass_kernel_spmd

B, C, S = 8, 512, 1024
H, DH, INNER = 8, 64, 512
EPS = 1e-5
SCALE = DH ** (-0.5)
N_CORES = 8
F32 = mybir.dt.float32
F32R = mybir.dt.float32r

_CACHE: dict = {}

KC = C // 128      # 4 contraction chunks over channels
IT = INNER // 128  # 4 tiles over inner dim (also head-pair index)
ST = S // 128      # 8 t-chunks
NSLAB = S // 512   # 2 s-slabs


def build_bass():
    nc = bacc.Bacc("TRN2", target_bir_lowering=False, debug=False,
                   num_devices=N_CORES)

    x_d = nc.dram_tensor("x", [C, S], F32, kind="ExternalInput")
    wqT_d = nc.dram_tensor("wqT", [C, INNER], F32, kind="ExternalInput")
    wkT_d = nc.dram_tensor("wkT", [C, INNER], F32, kind="ExternalInput")
    wvT_d = nc.dram_tensor("wvT", [C, INNER], F32, kind="ExternalInput")
    woT_d = nc.dram_tensor("woT", [INNER, C], F32, kind="ExternalInput")
    # bq | bk | bo packed on host as [128, 12] (col t+0/4/8 = vec[t*128+p])
    bias_d = nc.dram_tensor("bias_pack", [128, 3 * IT], F32, kind="ExternalInput")
    y_d = nc.dram_tensor("y", [C, S], F32, kind="ExternalOutput")

    with tile.TileContext(nc) as tc:
        with (
            tc.tile_pool(name="persist", bufs=1) as persist,
            tc.tile_pool(name="stage", bufs=2) as stage,
            tc.tile_pool(name="out", bufs=3) as outp,
            tc.tile_pool(name="et", bufs=4) as etp,
            tc.tile_pool(name="norm", bufs=2) as normp,
            tc.tile_pool(name="psP", bufs=2, space="PSUM") as psP,
            tc.tile_pool(name="psS", bufs=2, space="PSUM") as psS,
            tc.tile_pool(name="psO", bufs=1, space="PSUM") as psO,
        ):
            # ---- loads. Priority: wq first (alone on the sync/HWDGE queue,
            # so it lands in ~3us and the first projections start ASAP), the
            # remaining weights chained behind it in need-order (wv before wk:
            # attention needs V ready early). x + tiny biases ride the gpsimd
            # (SWDGE) queue in parallel. NOTHING on the scalar queue — the
            # scalar engine must stay exp-only. ----
            from concourse.tile import add_dep_helper

            # x chunks first (each its own tile so the casts pipeline), then
            # weights in need-order, all chained on the sync/HWDGE queue so
            # bandwidth goes to the critical transfer instead of round-robin.
            x_st = [stage.tile([128, 2, S], F32, tag=f"x_st{k}", name=f"x_st{k}",
                               bufs=1) for k in range(2)]
            dmas = []
            for h in range(2):
                i = nc.sync.dma_start(
                    x_st[h][:],
                    x_d.rearrange("(k p) s -> p k s", p=128)[:, 2 * h:2 * h + 2, :])
                dmas.append(i)
            wq_st = stage.tile([128, KC, 512], F32, tag="wq_st", bufs=1)
            dmas.append(nc.sync.dma_start(
                wq_st[:], wqT_d.rearrange("(k p) i -> p k i", p=128)))
            wk_st = stage.tile([128, KC, 512], F32, tag="wk_st", bufs=1)
            dmas.append(nc.sync.dma_start(
                wk_st[:], wkT_d.rearrange("(k p) i -> p k i", p=128)))
            wv_st = stage.tile([128, KC, 512], F32, tag="wv_st", bufs=1)
            dmas.append(nc.sync.dma_start(
                wv_st[:], wvT_d.rearrange("(k p) i -> p k i", p=128)))
            wo_st = stage.tile([128, KC, 512], F32, tag="wo_st", bufs=1)
            dmas.append(nc.sync.dma_start(
                wo_st[:], woT_d.rearrange("(k p) i -> p k i", p=128)))
            for a, b in zip(dmas[1:], dmas):
                add_dep_helper(a.ins, b.ins, sync=False, reason="dma priority")

            bias_sb = persist.tile([128, 3 * IT], F32, tag="bias")
            nc.gpsimd.dma_start(bias_sb[:], bias_d[:])
            bq_sb = bias_sb[:, 0:IT]
            bk_sb = bias_sb[:, IT:2 * IT]
            bo_sb = bias_sb[:, 2 * IT:3 * IT]

            # ---- casts to fp32r ----
            xr = persist.tile([128, KC, S], F32R, tag="xr", name="xr")
            for kc in range(KC):
                nc.vector.tensor_copy(xr[:, kc, :], x_st[kc // 2][:, kc % 2, :])
            wqr = persist.tile([128, KC, 512], F32R, tag="wqr", name="wqr")
            nc.vector.tensor_copy(wqr[:], wq_st[:])
            wkr = persist.tile([128, KC, 512], F32R, tag="wkr", name="wkr")
            nc.vector.tensor_copy(wkr[:], wk_st[:])
            wvr = persist.tile([128, KC, 512], F32R, tag="wvr", name="wvr")
            nc.vector.tensor_copy(wvr[:], wv_st[:])
            wor = persist.tile([128, KC, 512], F32R, tag="wor", name="wor")
            nc.vector.tensor_copy(wor[:], wo_st[:])

            ones_sb = persist.tile([128, H], F32, tag="ones")
            nc.vector.memset(ones_sb[:], 1.0)

            # ---- persistent per-slab outputs ----
            qT = [[persist.tile([128, 512], F32R, tag=f"qT{i}{s}",
                                name=f"qT{i}{s}") for s in range(NSLAB)]
                  for i in range(IT)]
            kT = [[persist.tile([128, 512], F32R, tag=f"kT{i}{s}",
                                name=f"kT{i}{s}") for s in range(NSLAB)]
                  for i in range(IT)]
            oT = [[persist.tile([128, 512], F32R, tag=f"oT{i}{s}",
                                name=f"oT{i}{s}") for s in range(NSLAB)]
                  for i in range(IT)]
            v_sb = [persist.tile([128, H * 65], F32R, tag=f"v{t}",
                                 name=f"v{t}") for t in range(ST)]

            def qk_piece(w, bias, dst, hp, sl):
                ps = psP.tile([128, 512], F32, tag="psP", name="psP")
                for kc in range(KC):
                    nc.tensor.matmul(
                        ps[:],
                        w[:, kc, hp * 128:(hp + 1) * 128],
                        xr[:, kc, sl * 512:(sl + 1) * 512],
                        start=(kc == 0), stop=(kc == KC - 1),
                    )
                nc.vector.tensor_scalar_add(
                    dst[hp][sl][:], ps[:], bias[:, hp:hp + 1]
                )

            def qk_proj_pieces(hp):
                return [
                    (lambda w=w, bias=bias, dst=dst, sl=sl:
                     qk_piece(w, bias, dst, hp, sl))
                    for w, bias, dst in ((wqr, bq_sb, qT), (wkr, bk_sb, kT))
                    for sl in range(NSLAB)
                ]

            def qk_proj(hp):
                for piece in qk_proj_pieces(hp):
                    piece()

            def v_proj(tc_):
                ps = psP.tile([128, 512], F32, tag="psP", name="psP")
                for kc in range(KC):
                    nc.tensor.matmul(
                        ps[:],
                        xr[:, kc, tc_ * 128:(tc_ + 1) * 128],
                        wvr[:, kc, :],
                        start=(kc == 0), stop=(kc == KC - 1),
                    )
                vv = v_sb[tc_][:].rearrange("p (h m) -> p h m", h=H)
                nc.vector.tensor_copy(
                    vv[:, :, 0:64], ps[:].rearrange("p (h m) -> p h m", h=H)
                )
                nc.vector.tensor_copy(vv[:, :, 64:65], ones_sb[:, :, None])

            def attention(sl, hp, fillers=()):
                fillers = list(fillers)
                h0, h1 = 2 * hp, 2 * hp + 1
                po0 = psO.tile([65, 512], F32, tag="po0", name="po0")
                po1 = psO.tile([65, 512], F32, tag="po1", name="po1")
                for tc_ in range(ST):
                    if tc_ % 2 == 1 and fillers:
                        fillers.pop(0)()
                    ksl, kcol = tc_ // 4, (tc_ % 4) * 128
                    pss = psS.tile([128, 1024], F32, tag="psS", name="psS")
                    nc.tensor.matmul(
                        pss[:, 0:512],
                        kT[hp][ksl][0:64, kcol:kcol + 128],
                        qT[hp][sl][0:64, :],
                        start=True, stop=True, tile_position=(0, 0),
                    )
                    nc.tensor.matmul(
                        pss[:, 512:1024],
                        kT[hp][ksl][64:128, kcol:kcol + 128],
                        qT[hp][sl][64:128, :],
                        start=True, stop=True, tile_position=(64, 0),
                    )
                    et = etp.tile([128, 1024], F32R, tag="et", name="et")
                    nc.scalar.activation(
                        et[:], pss[:], mybir.ActivationFunctionType.Exp
                    )
                    nc.tensor.matmul(
                        po0[:], v_sb[tc_][:, h0 * 65:(h0 + 1) * 65],
                        et[:, 0:512],
                        start=(tc_ == 0), stop=(tc_ == ST - 1),
                    )
                    nc.tensor.matmul(
                        po1[:], v_sb[tc_][:, h1 * 65:(h1 + 1) * 65],
                        et[:, 512:1024],
                        start=(tc_ == 0), stop=(tc_ == ST - 1),
                    )
                for half, po in ((0, po0), (1, po1)):
                    drow = normp.tile([1, 512], F32, tag="drow", name="drow")
                    nc.vector.tensor_copy(drow[:], po[64:65, :])
                    rrow = normp.tile([1, 512], F32, tag="rrow", name="rrow")
                    nc.vector.reciprocal_approx_fast(rrow[:], drow[:])
                    rbc = normp.tile([64, 512], F32, tag="rbc", name="rbc")
                    nc.gpsimd.partition_broadcast(rbc[:], rrow[:])
                    nc.vector.tensor_mul(
                        oT[hp][sl][half * 64:(half + 1) * 64, :],
                        po[0:64, :],
                        rbc[:],
                    )

            def out_proj_piece(sl, ct):
                ps = psP.tile([128, 512], F32, tag="psP", name="psP")
                for ic in range(IT):
                    nc.tensor.matmul(
                        ps[:],
                        wor[:, ic, ct * 128:(ct + 1) * 128],
                        oT[ic][sl][:],
                        start=(ic == 0), stop=(ic == IT - 1),
                    )
                ysb = outp.tile([128, 512], F32, tag="ysb", name="ysb")
                nc.vector.tensor_scalar_add(ysb[:], ps[:], bo_sb[:, ct:ct + 1])
                nc.sync.dma_start(
                    y_d[ct * 128:(ct + 1) * 128, sl * 512:(sl + 1) * 512],
                    ysb[:],
                )

            def out_proj_pieces(sl):
                return [
                    (lambda ct=ct: out_proj_piece(sl, ct)) for ct in range(IT)
                ]

            # ---- emission order (priority hint for the scheduler):
            # projections for head-pair hp+1 and the slab-0 output projection
            # are sprinkled between attention chunks so the PE fills the
            # slack of the ACT-bound attention inner loop instead of
            # monopolizing it in blocks. ----
            qk_proj(0)
            for tc_ in range(ST):
                v_proj(tc_)
            attention(0, 0, qk_proj_pieces(1))
            attention(0, 1, qk_proj_pieces(2))
            attention(0, 2, qk_proj_pieces(3))
            attention(0, 3)
            attention(1, 0, out_proj_pieces(0))
            attention(1, 1)
            attention(1, 2)
            attention(1, 3)
            for piece in out_proj_pieces(1):
                piece()

    nc.compile()
    return nc


def prep_host(inputs):
    """Fold BN + scale + v-bias into effective weights (fp32 numpy)."""
    x = np.asarray(inputs["x"], dtype=np.float32)
    g = np.asarray(inputs["bn_gamma"], dtype=np.float32)
    be = np.asarray(inputs["bn_beta"], dtype=np.float32)
    mu = np.asarray(inputs["bn_mean"], dtype=np.float32)
    var = np.asarray(inputs["bn_var"], dtype=np.float32)
    wq = np.asarray(inputs["wq"], dtype=np.float32)
    bq = np.asarray(inputs["bq"], dtype=np.float32)
    wk = np.asarray(inputs["wk"], dtype=np.float32)
    bk = np.asarray(inputs["bk"], dtype=np.float32)
    wv = np.asarray(inputs["wv"], dtype=np.float32)
    bv = np.asarray(inputs["bv"], dtype=np.float32)
    wo = np.asarray(inputs["wo"], dtype=np.float32)
    bo = np.asarray(inputs["bo"], dtype=np.float32)

    a = g / np.sqrt(var + EPS)          # [C]
    bvec = be - mu * a                  # [C]

    wq_eff = wq * a[None, :] * SCALE
    bq_eff = (bq + wq @ bvec) * SCALE
    wk_eff = wk * a[None, :]
    bk_eff = bk + wk @ bvec
    wv_eff = wv * a[None, :]
    bv_eff = bv + wv @ bvec
    bo_eff = bo + wo @ bv_eff           # v bias rides through softmax (sums to 1)

    bias_pack = np.concatenate(
        [bq_eff.reshape(IT, 128).T, bk_eff.reshape(IT, 128).T,
         bo_eff.reshape(IT, 128).T], axis=1
    ).astype(np.float32)

    per_core = []
    for b in range(B):
        per_core.append({
            "x": np.ascontiguousarray(x[b, :, :, 0]),
            "wqT": np.ascontiguousarray(wq_eff.T),
            "wkT": np.ascontiguousarray(wk_eff.T),
            "wvT": np.ascontiguousarray(wv_eff.T),
            "woT": np.ascontiguousarray(wo.T),
            "bias_pack": np.ascontiguousarray(bias_pack),
        })
    return per_core


def kernel(**inputs):
    if "nc" not in _CACHE:
        _CACHE["nc"] = build_bass()
    nc = _CACHE["nc"]
    in_maps = prep_host(inputs)
    res = run_bass_kernel_spmd(nc, in_maps, list(range(N_CORES)))
    y = np.stack([res.results[c]["y"] for c in range(N_CORES)], axis=0)
    return y[..., None].astype(np.float32)


def run_traced(**inputs):
    """Like kernel() but with NTFF profiling; returns (y, results, tmpdir)."""
    if "nc" not in _CACHE:
        _CACHE["nc"] = build_bass()
    nc = _CACHE["nc"]
    in_maps = prep_host(inputs)
    import tempfile
    tmpdir = tempfile.mkdtemp(prefix="mha_trace_")
    res = run_bass_kernel_spmd(
        nc, in_maps, list(range(N_CORES)), trace=True, tmpdir=tmpdir
    )
    y = np.stack([res.results[c]["y"] for c in range(N_CORES)], axis=0)
    return y[..., None].astype(np.float32), res, tmpdir


# revision 17
# speedup vs baseline: 1.1555x; 1.1076x over previous
"""Trainium2 Bass kernel for BatchNorm2d + 8-head self-attention block.

Reference (per batch element b, all fp32):
    xn = BN_eval(x[b]); t = xn.T
    q/k/v = t @ W.T + b            # [S, 512], 8 heads x 64
    attn  = softmax(q k^T / 8)     # per head
    y[b]  = ((attn v) @ wo.T + bo).T

Sharding: pure data parallel — one batch element per NeuronCore, weights
replicated, no collectives.

Device design (per core), fully in the "transposed" domain (no large
transposes anywhere):
  - BN folded into QKV weights/biases on host; 1/8 scale folded into wq/bq;
    v bias folded into bo (softmax rows sum to 1).
  - Q^T,K^T [I,S] = wT.T @ x      (x arrives [C,S] — natural rhs)
  - V [S,I]       = x_chunk.T @ wvT, stored interleaved per head with a
    ones column ([128, 8*65]) so the PV matmul (M=65) also produces the
    softmax denominators for free.
  - scores^T per head [t,s]; head pairs row-packed via tile_position
    (0,0)/(64,0), K=64 each; exp on ScalarE over both heads in one call
    (no max subtraction — scores are in [-3, 3]).
  - o^T accumulates over 8 t-chunks (K=128); normalize = approx-reciprocal
    row + gpsimd partition-broadcast + DVE multiply; y^T = woT.T @ o^T + bo.

Matmul dtype is fp16 by default: 2-byte weights keep LDWEIGHTS in the PE's
background buffer (hidden behind the previous matmul) where 4-byte fp32r
weights serialize ~150ns per matmul; fp16's 10-bit mantissa keeps the end
to-end error ~1e-3 of scale (all activations are within [-20, 20]).
Set dt_mm=float32r for a ~1e-4-accuracy variant (~1.4x slower).
"""

import numpy as np

import concourse.bass as bass
import concourse.tile as tile
from concourse import bacc, mybir
from concourse.bass_utils import run_bass_kernel_spmd
from concourse.tile import add_dep_helper

B, C, S = 8, 512, 1024
H, DH, INNER = 8, 64, 512
EPS = 1e-5
SCALE = DH ** (-0.5)
N_CORES = 8
F32 = mybir.dt.float32
F32R = mybir.dt.float32r
F16 = mybir.dt.float16

DT_MM = F16  # matmul dtype: F16 (fast) or F32R (precise)

_CACHE: dict = {}

KC = C // 128      # 4 contraction chunks over channels
IT = INNER // 128  # 4 tiles over inner dim (also head-pair index)
ST = S // 128      # 8 t-chunks
NSLAB = S // 512   # 2 s-slabs


def build_bass(dt_mm):
    two_byte = mybir.dt.size(dt_mm) == 2
    dt_in = dt_mm if two_byte else F32
    nc = bacc.Bacc("TRN2", target_bir_lowering=False, debug=False,
                   num_devices=N_CORES)

    x_d = nc.dram_tensor("x", [C, S], dt_in, kind="ExternalInput")
    wqT_d = nc.dram_tensor("wqT", [C, INNER], dt_in, kind="ExternalInput")
    wkT_d = nc.dram_tensor("wkT", [C, INNER], dt_in, kind="ExternalInput")
    wvT_d = nc.dram_tensor("wvT", [C, INNER], dt_in, kind="ExternalInput")
    woT_d = nc.dram_tensor("woT", [INNER, C], dt_in, kind="ExternalInput")
    # bq | bk | bo packed on host as [128, 12] (col t+0/4/8 = vec[t*128+p])
    bias_d = nc.dram_tensor("bias_pack", [128, 3 * IT], F32, kind="ExternalInput")
    y_d = nc.dram_tensor("y", [C, S], F32, kind="ExternalOutput")

    with tile.TileContext(nc) as tc:
        with (
            tc.tile_pool(name="persist", bufs=1) as persist,
            tc.tile_pool(name="stage", bufs=2) as stage,
            tc.tile_pool(name="out", bufs=3) as outp,
            tc.tile_pool(name="et", bufs=4) as etp,
            tc.tile_pool(name="norm", bufs=2) as normp,
            tc.tile_pool(name="psP", bufs=2, space="PSUM") as psP,
            tc.tile_pool(name="psS", bufs=2, space="PSUM") as psS,
            tc.tile_pool(name="psO", bufs=1, space="PSUM") as psO,
        ):
            # ---- loads, chained on the sync/HWDGE queue in need-order so
            # bandwidth goes to the critical transfer instead of round-robin;
            # tiny bias pack rides the gpsimd/SWDGE queue in parallel ----
            xr = persist.tile([128, KC, S], dt_mm, tag="xr", name="xr")
            wqr = persist.tile([128, KC, 512], dt_mm, tag="wqr", name="wqr")
            wkr = persist.tile([128, KC, 512], dt_mm, tag="wkr", name="wkr")
            wvr = persist.tile([128, KC, 512], dt_mm, tag="wvr", name="wvr")
            wor = persist.tile([128, KC, 512], dt_mm, tag="wor", name="wor")

            dmas = []
            if two_byte:
                # DMA straight into the matmul tiles — no casts needed.
                x_r3 = x_d.rearrange("(k p) s -> p k s", p=128)
                for h in range(2):
                    dmas.append(nc.sync.dma_start(
                        xr[:, 2 * h:2 * h + 2, :], x_r3[:, 2 * h:2 * h + 2, :]))
                for dst, src in ((wqr, wqT_d), (wkr, wkT_d), (wvr, wvT_d),
                                 (wor, woT_d)):
                    dmas.append(nc.sync.dma_start(
                        dst[:], src.rearrange("(k p) i -> p k i", p=128)))
            else:
                x_st = [stage.tile([128, 2, S], F32, tag=f"x_st{k}",
                                   name=f"x_st{k}", bufs=1) for k in range(2)]
                for h in range(2):
                    dmas.append(nc.sync.dma_start(
                        x_st[h][:],
                        x_d.rearrange("(k p) s -> p k s", p=128)[:, 2 * h:2 * h + 2, :]))
                w_st = {}
                for nm, src in (("wq", wqT_d), ("wk", wkT_d), ("wv", wvT_d),
                                ("wo", woT_d)):
                    st = stage.tile([128, KC, 512], F32, tag=f"{nm}_st",
                                    name=f"{nm}_st", bufs=1)
                    w_st[nm] = st
                    dmas.append(nc.sync.dma_start(
                        st[:], src.rearrange("(k p) i -> p k i", p=128)))
            for a, b in zip(dmas[1:], dmas):
                add_dep_helper(a.ins, b.ins, sync=False, reason="dma priority")

            bias_sb = persist.tile([128, 3 * IT], F32, tag="bias")
            nc.gpsimd.dma_start(bias_sb[:], bias_d[:])
            bq_sb = bias_sb[:, 0:IT]
            bk_sb = bias_sb[:, IT:2 * IT]
            bo_sb = bias_sb[:, 2 * IT:3 * IT]

            if not two_byte:
                for kc in range(KC):
                    nc.vector.tensor_copy(xr[:, kc, :], x_st[kc // 2][:, kc % 2, :])
                nc.vector.tensor_copy(wqr[:], w_st["wq"][:])
                nc.vector.tensor_copy(wkr[:], w_st["wk"][:])
                nc.vector.tensor_copy(wvr[:], w_st["wv"][:])
                nc.vector.tensor_copy(wor[:], w_st["wo"][:])

            ones_sb = persist.tile([128, H], F32, tag="ones")
            nc.vector.memset(ones_sb[:], 1.0)

            # ---- persistent per-slab outputs ----
            qT = [[persist.tile([128, 512], dt_mm, tag=f"qT{i}{s}",
                                name=f"qT{i}{s}") for s in range(NSLAB)]
                  for i in range(IT)]
            kT = [[persist.tile([128, 512], dt_mm, tag=f"kT{i}{s}",
                                name=f"kT{i}{s}") for s in range(NSLAB)]
                  for i in range(IT)]
            oT = [[persist.tile([128, 512], dt_mm, tag=f"oT{i}{s}",
                                name=f"oT{i}{s}") for s in range(NSLAB)]
                  for i in range(IT)]
            v_sb = [persist.tile([128, H * 65], dt_mm, tag=f"v{t}",
                                 name=f"v{t}") for t in range(ST)]

            def qk_piece(w, bias, dst, hp, sl):
                ps = psP.tile([128, 512], F32, tag="psP", name="psP")
                for kc in range(KC):
                    nc.tensor.matmul(
                        ps[:],
                        w[:, kc, hp * 128:(hp + 1) * 128],
                        xr[:, kc, sl * 512:(sl + 1) * 512],
                        start=(kc == 0), stop=(kc == KC - 1),
                    )
                nc.vector.tensor_scalar_add(
                    dst[hp][sl][:], ps[:], bias[:, hp:hp + 1]
                )

            def qk_proj_pieces(hp):
                return [
                    (lambda w=w, bias=bias, dst=dst, sl=sl:
                     qk_piece(w, bias, dst, hp, sl))
                    for w, bias, dst in ((wqr, bq_sb, qT), (wkr, bk_sb, kT))
                    for sl in range(NSLAB)
                ]

            def qk_proj(hp):
                for piece in qk_proj_pieces(hp):
                    piece()

            def v_proj(tc_):
                ps = psP.tile([128, 512], F32, tag="psP", name="psP")
                for kc in range(KC):
                    nc.tensor.matmul(
                        ps[:],
                        xr[:, kc, tc_ * 128:(tc_ + 1) * 128],
                        wvr[:, kc, :],
                        start=(kc == 0), stop=(kc == KC - 1),
                    )
                vv = v_sb[tc_][:].rearrange("p (h m) -> p h m", h=H)
                nc.vector.tensor_copy(
                    vv[:, :, 0:64], ps[:].rearrange("p (h m) -> p h m", h=H)
                )
                nc.vector.tensor_copy(vv[:, :, 64:65], ones_sb[:, :, None])

            def attention(sl, hp, fillers=()):
                fillers = list(fillers)
                h0, h1 = 2 * hp, 2 * hp + 1
                po0 = psO.tile([65, 512], F32, tag="po0", name="po0")
                po1 = psO.tile([65, 512], F32, tag="po1", name="po1")
                for tc_ in range(ST):
                    if tc_ % 2 == 1 and fillers:
                        fillers.pop(0)()
                    ksl, kcol = tc_ // 4, (tc_ % 4) * 128
                    pss = psS.tile([128, 1024], F32, tag="psS", name="psS")
                    nc.tensor.matmul(
                        pss[:, 0:512],
                        kT[hp][ksl][0:64, kcol:kcol + 128],
                        qT[hp][sl][0:64, :],
                        start=True, stop=True, tile_position=(0, 0),
                    )
                    nc.tensor.matmul(
                        pss[:, 512:1024],
                        kT[hp][ksl][64:128, kcol:kcol + 128],
                        qT[hp][sl][64:128, :],
                        start=True, stop=True, tile_position=(64, 0),
                    )
                    et = etp.tile([128, 1024], dt_mm, tag="et", name="et")
                    nc.scalar.activation(
                        et[:], pss[:], mybir.ActivationFunctionType.Exp
                    )
                    nc.tensor.matmul(
                        po0[:], v_sb[tc_][:, h0 * 65:(h0 + 1) * 65],
                        et[:, 0:512],
                        start=(tc_ == 0), stop=(tc_ == ST - 1),
                    )
                    nc.tensor.matmul(
                        po1[:], v_sb[tc_][:, h1 * 65:(h1 + 1) * 65],
                        et[:, 512:1024],
                        start=(tc_ == 0), stop=(tc_ == ST - 1),
                    )
                for half, po in ((0, po0), (1, po1)):
                    drow = normp.tile([1, 512], F32, tag="drow", name="drow")
                    nc.vector.tensor_copy(drow[:], po[64:65, :])
                    rrow = normp.tile([1, 512], F32, tag="rrow", name="rrow")
                    nc.vector.reciprocal_approx_fast(rrow[:], drow[:])
                    rbc = normp.tile([64, 512], F32, tag="rbc", name="rbc")
                    nc.gpsimd.partition_broadcast(rbc[:], rrow[:])
                    nc.vector.tensor_mul(
                        oT[hp][sl][half * 64:(half + 1) * 64, :],
                        po[0:64, :],
                        rbc[:],
                    )

            def out_proj_piece(sl, ct):
                ps = psP.tile([128, 512], F32, tag="psP", name="psP")
                for ic in range(IT):
                    nc.tensor.matmul(
                        ps[:],
                        wor[:, ic, ct * 128:(ct + 1) * 128],
                        oT[ic][sl][:],
                        start=(ic == 0), stop=(ic == IT - 1),
                    )
                ysb = outp.tile([128, 512], F32, tag="ysb", name="ysb")
                nc.vector.tensor_scalar_add(ysb[:], ps[:], bo_sb[:, ct:ct + 1])
                nc.sync.dma_start(
                    y_d[ct * 128:(ct + 1) * 128, sl * 512:(sl + 1) * 512],
                    ysb[:],
                )

            def out_proj_pieces(sl):
                return [
                    (lambda ct=ct: out_proj_piece(sl, ct)) for ct in range(IT)
                ]

            # ---- emission order (priority hint for the scheduler):
            # projections for head-pair hp+1 and the slab-0 output projection
            # are sprinkled between attention chunks so the PE fills the
            # slack of the exp-paced attention loop instead of monopolizing
            # it in blocks. ----
            qk_proj(0)
            for tc_ in range(ST):
                v_proj(tc_)
            attention(0, 0, qk_proj_pieces(1))
            attention(0, 1, qk_proj_pieces(2))
            attention(0, 2, qk_proj_pieces(3))
            attention(0, 3)
            attention(1, 0, out_proj_pieces(0))
            attention(1, 1)
            attention(1, 2)
            attention(1, 3)
            for piece in out_proj_pieces(1):
                piece()

    nc.compile()
    return nc


def prep_host(inputs, dt_mm):
    """Fold BN + scale + v-bias into effective weights (fp32 numpy)."""
    x = np.asarray(inputs["x"], dtype=np.float32)
    g = np.asarray(inputs["bn_gamma"], dtype=np.float32)
    be = np.asarray(inputs["bn_beta"], dtype=np.float32)
    mu = np.asarray(inputs["bn_mean"], dtype=np.float32)
    var = np.asarray(inputs["bn_var"], dtype=np.float32)
    wq = np.asarray(inputs["wq"], dtype=np.float32)
    bq = np.asarray(inputs["bq"], dtype=np.float32)
    wk = np.asarray(inputs["wk"], dtype=np.float32)
    bk = np.asarray(inputs["bk"], dtype=np.float32)
    wv = np.asarray(inputs["wv"], dtype=np.float32)
    bv = np.asarray(inputs["bv"], dtype=np.float32)
    wo = np.asarray(inputs["wo"], dtype=np.float32)
    bo = np.asarray(inputs["bo"], dtype=np.float32)

    a = g / np.sqrt(var + EPS)          # [C]
    bvec = be - mu * a                  # [C]

    wq_eff = wq * a[None, :] * SCALE
    bq_eff = (bq + wq @ bvec) * SCALE
    wk_eff = wk * a[None, :]
    bk_eff = bk + wk @ bvec
    wv_eff = wv * a[None, :]
    bv_eff = bv + wv @ bvec
    bo_eff = bo + wo @ bv_eff           # v bias rides through softmax (sums to 1)

    bias_pack = np.concatenate(
        [bq_eff.reshape(IT, 128).T, bk_eff.reshape(IT, 128).T,
         bo_eff.reshape(IT, 128).T], axis=1
    ).astype(np.float32)

    np_dt = np.float16 if mybir.dt.size(dt_mm) == 2 else np.float32
    per_core = []
    for b in range(B):
        per_core.append({
            "x": np.ascontiguousarray(x[b, :, :, 0].astype(np_dt)),
            "wqT": np.ascontiguousarray(wq_eff.T.astype(np_dt)),
            "wkT": np.ascontiguousarray(wk_eff.T.astype(np_dt)),
            "wvT": np.ascontiguousarray(wv_eff.T.astype(np_dt)),
            "woT": np.ascontiguousarray(wo.T.astype(np_dt)),
            "bias_pack": np.ascontiguousarray(bias_pack),
        })
    return per_core


def _get_nc(dt_mm):
    key = str(dt_mm)
    if key not in _CACHE:
        _CACHE[key] = build_bass(dt_mm)
    return _CACHE[key]


def kernel(**inputs):
    nc = _get_nc(DT_MM)
    in_maps = prep_host(inputs, DT_MM)
    res = run_bass_kernel_spmd(nc, in_maps, list(range(N_CORES)))
    y = np.stack([res.results[c]["y"] for c in range(N_CORES)], axis=0)
    return y[..., None].astype(np.float32)


def run_traced(**inputs):
    """Like kernel() but with NTFF profiling; returns (y, results, tmpdir)."""
    nc = _get_nc(DT_MM)
    in_maps = prep_host(inputs, DT_MM)
    import tempfile
    tmpdir = tempfile.mkdtemp(prefix="mha_trace_")
    res = run_bass_kernel_spmd(
        nc, in_maps, list(range(N_CORES)), trace=True, tmpdir=tmpdir
    )
    y = np.stack([res.results[c]["y"] for c in range(N_CORES)], axis=0)
    return y[..., None].astype(np.float32), res, tmpdir


# revision 20
# speedup vs baseline: 1.2453x; 1.0778x over previous
"""Trainium2 Bass kernel for BatchNorm2d + 8-head self-attention block.

Reference (per batch element b, all fp32):
    xn = BN_eval(x[b]); t = xn.T
    q/k/v = t @ W.T + b            # [S, 512], 8 heads x 64
    attn  = softmax(q k^T / 8)     # per head
    y[b]  = ((attn v) @ wo.T + bo).T

Sharding: pure data parallel — one batch element per NeuronCore, weights
replicated, no collectives.

Device design (per core), fully in the "transposed" domain (no large
transposes anywhere):
  - BN folded into QKV weights/biases on host; 1/8 scale folded into wq/bq;
    v bias folded into bo (softmax rows sum to 1).
  - Q^T,K^T [I,S] = wT.T @ x      (x arrives [C,S] — natural rhs)
  - V [S,I]       = x_chunk.T @ wvT, stored interleaved per head with a
    ones column ([128, 8*65]) so the PV matmul (M=65) also produces the
    softmax denominators for free.
  - scores^T per head [t,s]; head pairs row-packed via tile_position
    (0,0)/(64,0), K=64 each; exp on ScalarE over both heads in one call
    (no max subtraction — scores are in [-3, 3]).
  - o^T accumulates over 8 t-chunks (K=128); normalize = approx-reciprocal
    row + gpsimd partition-broadcast + DVE multiply; y^T = woT.T @ o^T + bo.

Matmul dtype is fp16 by default: 2-byte weights keep LDWEIGHTS in the PE's
background buffer (hidden behind the previous matmul) where 4-byte fp32r
weights serialize ~150ns per matmul; fp16's 10-bit mantissa keeps the end
to-end error ~1e-3 of scale (all activations are within [-20, 20]).
Set dt_mm=float32r for a ~1e-4-accuracy variant (~1.4x slower).
"""

import numpy as np

import concourse.bass as bass
import concourse.tile as tile
from concourse import bacc, mybir
from concourse.bass_utils import run_bass_kernel_spmd
from concourse.tile import add_dep_helper

B, C, S = 8, 512, 1024
H, DH, INNER = 8, 64, 512
EPS = 1e-5
SCALE = DH ** (-0.5)
N_CORES = 8
F32 = mybir.dt.float32
F32R = mybir.dt.float32r
F16 = mybir.dt.float16

DT_MM = F16  # matmul dtype: F16 (fast) or F32R (precise)

_CACHE: dict = {}

KC = C // 128      # 4 contraction chunks over channels
IT = INNER // 128  # 4 tiles over inner dim (also head-pair index)
ST = S // 128      # 8 t-chunks
NSLAB = S // 512   # 2 s-slabs


def build_bass(dt_mm):
    two_byte = mybir.dt.size(dt_mm) == 2
    dt_in = dt_mm if two_byte else F32
    nc = bacc.Bacc("TRN2", target_bir_lowering=False, debug=False,
                   num_devices=N_CORES)

    x_d = nc.dram_tensor("x", [C, S], dt_in, kind="ExternalInput")
    wqT_d = nc.dram_tensor("wqT", [C, INNER], dt_in, kind="ExternalInput")
    wkT_d = nc.dram_tensor("wkT", [C, INNER], dt_in, kind="ExternalInput")
    wvT_d = nc.dram_tensor("wvT", [C, INNER], dt_in, kind="ExternalInput")
    woT_d = nc.dram_tensor("woT", [INNER, C], dt_in, kind="ExternalInput")
    # bq | bk | bo packed on host as [128, 12] (col t+0/4/8 = vec[t*128+p])
    bias_d = nc.dram_tensor("bias_pack", [128, 3 * IT], F32, kind="ExternalInput")
    y_d = nc.dram_tensor("y", [C, S], F32, kind="ExternalOutput")

    with tile.TileContext(nc) as tc:
        with (
            tc.tile_pool(name="persist", bufs=1) as persist,
            tc.tile_pool(name="stage", bufs=2) as stage,
            tc.tile_pool(name="out", bufs=3) as outp,
            tc.tile_pool(name="et", bufs=4) as etp,
            tc.tile_pool(name="norm", bufs=2) as normp,
            # one shared 4-slot pool for every 1-bank accumulator (projection
            # groups AND the two attention po accumulators): a fresh bank is
            # always available at head-pair transitions, so the strict-FIFO
            # PE queue never stalls behind the normalize chain.
            tc.tile_pool(name="psA", bufs=4, space="PSUM") as psA,
            tc.tile_pool(name="psS", bufs=2, space="PSUM") as psS,
        ):
            # ---- loads, chained on the sync/HWDGE queue in need-order so
            # bandwidth goes to the critical transfer instead of round-robin;
            # tiny bias pack rides the gpsimd/SWDGE queue in parallel ----
            xr = persist.tile([128, KC, S], dt_mm, tag="xr", name="xr")
            wqr = persist.tile([128, KC, 512], dt_mm, tag="wqr", name="wqr")
            wkr = persist.tile([128, KC, 512], dt_mm, tag="wkr", name="wkr")
            wvr = persist.tile([128, KC, 512], dt_mm, tag="wvr", name="wvr")
            wor = persist.tile([128, KC, 512], dt_mm, tag="wor", name="wor")

            dmas = []
            if two_byte:
                # DMA straight into the matmul tiles — no casts needed.
                x_r3 = x_d.rearrange("(k p) s -> p k s", p=128)
                for h in range(2):
                    dmas.append(nc.sync.dma_start(
                        xr[:, 2 * h:2 * h + 2, :], x_r3[:, 2 * h:2 * h + 2, :]))
                for dst, src in ((wqr, wqT_d), (wkr, wkT_d), (wvr, wvT_d),
                                 (wor, woT_d)):
                    dmas.append(nc.sync.dma_start(
                        dst[:], src.rearrange("(k p) i -> p k i", p=128)))
            else:
                x_st = [stage.tile([128, 2, S], F32, tag=f"x_st{k}",
                                   name=f"x_st{k}", bufs=1) for k in range(2)]
                for h in range(2):
                    dmas.append(nc.sync.dma_start(
                        x_st[h][:],
                        x_d.rearrange("(k p) s -> p k s", p=128)[:, 2 * h:2 * h + 2, :]))
                w_st = {}
                for nm, src in (("wq", wqT_d), ("wk", wkT_d), ("wv", wvT_d),
                                ("wo", woT_d)):
                    st = stage.tile([128, KC, 512], F32, tag=f"{nm}_st",
                                    name=f"{nm}_st", bufs=1)
                    w_st[nm] = st
                    dmas.append(nc.sync.dma_start(
                        st[:], src.rearrange("(k p) i -> p k i", p=128)))
            for a, b in zip(dmas[1:], dmas):
                add_dep_helper(a.ins, b.ins, sync=False, reason="dma priority")

            bias_sb = persist.tile([128, 3 * IT], F32, tag="bias")
            nc.gpsimd.dma_start(bias_sb[:], bias_d[:])
            bq_sb = bias_sb[:, 0:IT]
            bk_sb = bias_sb[:, IT:2 * IT]
            bo_sb = bias_sb[:, 2 * IT:3 * IT]

            if not two_byte:
                for kc in range(KC):
                    nc.vector.tensor_copy(xr[:, kc, :], x_st[kc // 2][:, kc % 2, :])
                nc.vector.tensor_copy(wqr[:], w_st["wq"][:])
                nc.vector.tensor_copy(wkr[:], w_st["wk"][:])
                nc.vector.tensor_copy(wvr[:], w_st["wv"][:])
                nc.vector.tensor_copy(wor[:], w_st["wo"][:])

            ones_sb = persist.tile([128, H], F32, tag="ones")
            nc.vector.memset(ones_sb[:], 1.0)

            # ---- persistent per-slab outputs ----
            qT = [[persist.tile([128, 512], dt_mm, tag=f"qT{i}{s}",
                                name=f"qT{i}{s}") for s in range(NSLAB)]
                  for i in range(IT)]
            kT = [[persist.tile([128, 512], dt_mm, tag=f"kT{i}{s}",
                                name=f"kT{i}{s}") for s in range(NSLAB)]
                  for i in range(IT)]
            oT = [[persist.tile([128, 512], dt_mm, tag=f"oT{i}{s}",
                                name=f"oT{i}{s}") for s in range(NSLAB)]
                  for i in range(IT)]
            v_sb = [persist.tile([128, H * 65], dt_mm, tag=f"v{t}",
                                 name=f"v{t}") for t in range(ST)]

            def qk_piece(w, bias, dst, hp, sl):
                ps = psA.tile([128, 512], F32, tag="acc", name="acc")
                for kc in range(KC):
                    nc.tensor.matmul(
                        ps[:],
                        w[:, kc, hp * 128:(hp + 1) * 128],
                        xr[:, kc, sl * 512:(sl + 1) * 512],
                        start=(kc == 0), stop=(kc == KC - 1),
                    )
                nc.vector.tensor_scalar_add(
                    dst[hp][sl][:], ps[:], bias[:, hp:hp + 1]
                )

            def qk_proj_pieces(hp):
                return [
                    (lambda w=w, bias=bias, dst=dst, sl=sl:
                     qk_piece(w, bias, dst, hp, sl))
                    for w, bias, dst in ((wqr, bq_sb, qT), (wkr, bk_sb, kT))
                    for sl in range(NSLAB)
                ]

            def qk_proj(hp):
                for piece in qk_proj_pieces(hp):
                    piece()

            def v_proj(tc_):
                ps = psA.tile([128, 512], F32, tag="acc", name="acc")
                for kc in range(KC):
                    nc.tensor.matmul(
                        ps[:],
                        xr[:, kc, tc_ * 128:(tc_ + 1) * 128],
                        wvr[:, kc, :],
                        start=(kc == 0), stop=(kc == KC - 1),
                    )
                vv = v_sb[tc_][:].rearrange("p (h m) -> p h m", h=H)
                nc.vector.tensor_copy(
                    vv[:, :, 0:64], ps[:].rearrange("p (h m) -> p h m", h=H)
                )
                nc.vector.tensor_copy(vv[:, :, 64:65], ones_sb[:, :, None])

            def attention(sl, hp, fillers):
                h0, h1 = 2 * hp, 2 * hp + 1
                po0 = psA.tile([65, 512], F32, tag="acc", name="po0")
                po1 = psA.tile([65, 512], F32, tag="acc", name="po1")
                for tc_ in range(ST):
                    if fillers:
                        fillers.pop(0)()
                    ksl, kcol = tc_ // 4, (tc_ % 4) * 128
                    pss = psS.tile([128, 1024], F32, tag="psS", name="psS")
                    nc.tensor.matmul(
                        pss[:, 0:512],
                        kT[hp][ksl][0:64, kcol:kcol + 128],
                        qT[hp][sl][0:64, :],
                        start=True, stop=True, tile_position=(0, 0),
                    )
                    nc.tensor.matmul(
                        pss[:, 512:1024],
                        kT[hp][ksl][64:128, kcol:kcol + 128],
                        qT[hp][sl][64:128, :],
                        start=True, stop=True, tile_position=(64, 0),
                    )
                    et = etp.tile([128, 1024], dt_mm, tag="et", name="et")
                    nc.scalar.activation(
                        et[:], pss[:], mybir.ActivationFunctionType.Exp
                    )
                    nc.tensor.matmul(
                        po0[:], v_sb[tc_][:, h0 * 65:(h0 + 1) * 65],
                        et[:, 0:512],
                        start=(tc_ == 0), stop=(tc_ == ST - 1),
                    )
                    nc.tensor.matmul(
                        po1[:], v_sb[tc_][:, h1 * 65:(h1 + 1) * 65],
                        et[:, 512:1024],
                        start=(tc_ == 0), stop=(tc_ == ST - 1),
                    )
                for half, po in ((0, po0), (1, po1)):
                    drow = normp.tile([1, 512], F32, tag="drow", name="drow")
                    nc.vector.tensor_copy(drow[:], po[64:65, :])
                    rrow = normp.tile([1, 512], F32, tag="rrow", name="rrow")
                    nc.vector.reciprocal_approx_fast(rrow[:], drow[:])
                    rbc = normp.tile([64, 512], F32, tag="rbc", name="rbc")
                    nc.gpsimd.partition_broadcast(rbc[:], rrow[:])
                    nc.vector.tensor_mul(
                        oT[hp][sl][half * 64:(half + 1) * 64, :],
                        po[0:64, :],
                        rbc[:],
                    )

            def out_proj_piece(sl, ct):
                ps = psA.tile([128, 512], F32, tag="acc", name="acc")
                for ic in range(IT):
                    nc.tensor.matmul(
                        ps[:],
                        wor[:, ic, ct * 128:(ct + 1) * 128],
                        oT[ic][sl][:],
                        start=(ic == 0), stop=(ic == IT - 1),
                    )
                ysb = outp.tile([128, 512], F32, tag="ysb", name="ysb")
                nc.vector.tensor_scalar_add(ysb[:], ps[:], bo_sb[:, ct:ct + 1])
                nc.sync.dma_start(
                    y_d[ct * 128:(ct + 1) * 128, sl * 512:(sl + 1) * 512],
                    ysb[:],
                )

            def out_proj_pieces(sl):
                return [
                    (lambda ct=ct: out_proj_piece(sl, ct)) for ct in range(IT)
                ]

            # ---- emission order (priority hint for the scheduler):
            # projections for head-pair hp+1 and the slab-0 output projection
            # are sprinkled between attention chunks so the PE fills the
            # slack of the exp-paced attention loop instead of monopolizing
            # it in blocks. ----
            # filler schedules respect write-before-read program order:
            # qk pieces are (q,sl0)=a, (q,sl1)=b, (k,sl0)=c, (k,sl1)=d;
            # attention(0,hp) reads a,c of head-pair hp at its chunk 0 and d
            # at chunk 4, so a,c are emitted during the PREVIOUS call and d
            # at this call's chunk 0.
            qk_proj(0)
            v_proj(0)
            v_proj(1)
            qk1 = qk_proj_pieces(1)
            qk2 = qk_proj_pieces(2)
            qk3 = qk_proj_pieces(3)
            vs = [(lambda t=t: v_proj(t)) for t in range(2, ST)]
            attention(0, 0, vs + [qk1[0], qk1[2]])
            attention(0, 1, [qk1[3], qk1[1], qk2[0], qk2[2]])
            attention(0, 2, [qk2[3], qk2[1], qk3[0], qk3[2]])
            attention(0, 3, [qk3[3], qk3[1]])
            attention(1, 0, out_proj_pieces(0))
            attention(1, 1, [])
            attention(1, 2, [])
            attention(1, 3, [])
            for piece in out_proj_pieces(1):
                piece()

    nc.compile()
    return nc


def prep_host(inputs, dt_mm):
    """Fold BN + scale + v-bias into effective weights (fp32 numpy)."""
    x = np.asarray(inputs["x"], dtype=np.float32)
    g = np.asarray(inputs["bn_gamma"], dtype=np.float32)
    be = np.asarray(inputs["bn_beta"], dtype=np.float32)
    mu = np.asarray(inputs["bn_mean"], dtype=np.float32)
    var = np.asarray(inputs["bn_var"], dtype=np.float32)
    wq = np.asarray(inputs["wq"], dtype=np.float32)
    bq = np.asarray(inputs["bq"], dtype=np.float32)
    wk = np.asarray(inputs["wk"], dtype=np.float32)
    bk = np.asarray(inputs["bk"], dtype=np.float32)
    wv = np.asarray(inputs["wv"], dtype=np.float32)
    bv = np.asarray(inputs["bv"], dtype=np.float32)
    wo = np.asarray(inputs["wo"], dtype=np.float32)
    bo = np.asarray(inputs["bo"], dtype=np.float32)

    a = g / np.sqrt(var + EPS)          # [C]
    bvec = be - mu * a                  # [C]

    wq_eff = wq * a[None, :] * SCALE
    bq_eff = (bq + wq @ bvec) * SCALE
    wk_eff = wk * a[None, :]
    bk_eff = bk + wk @ bvec
    wv_eff = wv * a[None, :]
    bv_eff = bv + wv @ bvec
    bo_eff = bo + wo @ bv_eff           # v bias rides through softmax (sums to 1)

    bias_pack = np.concatenate(
        [bq_eff.reshape(IT, 128).T, bk_eff.reshape(IT, 128).T,
         bo_eff.reshape(IT, 128).T], axis=1
    ).astype(np.float32)

    np_dt = np.float16 if mybir.dt.size(dt_mm) == 2 else np.float32
    per_core = []
    for b in range(B):
        per_core.append({
            "x": np.ascontiguousarray(x[b, :, :, 0].astype(np_dt)),
            "wqT": np.ascontiguousarray(wq_eff.T.astype(np_dt)),
            "wkT": np.ascontiguousarray(wk_eff.T.astype(np_dt)),
            "wvT": np.ascontiguousarray(wv_eff.T.astype(np_dt)),
            "woT": np.ascontiguousarray(wo.T.astype(np_dt)),
            "bias_pack": np.ascontiguousarray(bias_pack),
        })
    return per_core


def _get_nc(dt_mm):
    key = str(dt_mm)
    if key not in _CACHE:
        _CACHE[key] = build_bass(dt_mm)
    return _CACHE[key]


def kernel(**inputs):
    nc = _get_nc(DT_MM)
    in_maps = prep_host(inputs, DT_MM)
    res = run_bass_kernel_spmd(nc, in_maps, list(range(N_CORES)))
    y = np.stack([res.results[c]["y"] for c in range(N_CORES)], axis=0)
    return y[..., None].astype(np.float32)


def run_traced(**inputs):
    """Like kernel() but with NTFF profiling; returns (y, results, tmpdir)."""
    nc = _get_nc(DT_MM)
    in_maps = prep_host(inputs, DT_MM)
    import tempfile
    tmpdir = tempfile.mkdtemp(prefix="mha_trace_")
    res = run_bass_kernel_spmd(
        nc, in_maps, list(range(N_CORES)), trace=True, tmpdir=tmpdir
    )
    y = np.stack([res.results[c]["y"] for c in range(N_CORES)], axis=0)
    return y[..., None].astype(np.float32), res, tmpdir
